# revision 21
# baseline (speedup 1.0000x reference)
"""Trainium2 Bass kernel for LocalSparseAttention.

Problem (hardcoded): B=2, S=2048, D=1024, H=16, HD=64, WINDOW=128 (band
|i-j| <= 64), fp32 I/O.

Sharding: 8 cores = 2 batches x 4 head-groups (4 heads each). Each core:
  - v projection first (seq-sliced xT DMA, priority-ordered), then qk
    projection into transposed layout [512, 2048] (head-pair packed)
  - banded attention: per 128-query tile, 256-key window; exp on ACT,
    0/1-mask multiply on DVE; AV with col-split M=64 (two heads in
    different PE col groups); denominator via ones[128,128] broadcast
    matmul (lands replicated on all 128 partitions), reciprocal on DVE
  - output projection -> fp16 partial [2048, 1024]
Host: fp16 casts + packed layouts in, sum of 4 partials per batch +
fused bias (b_out + b_v @ w_out) out.

All matmuls fp16 (1 cycle/row on PE) with fp32 PSUM accumulation.
"""
import sys

if "/opt/trn_rl_repo" not in sys.path:
    sys.path.insert(0, "/opt/trn_rl_repo")

import numpy as np

import concourse.bass as bass
import concourse.mybir as mybir
import concourse.tile as tile
from concourse import bacc
from concourse.bass_utils import run_bass_kernel_spmd

B, S, D, H, HD = 2, 2048, 1024, 16, 64
SCALE = HD**-0.5
C_SUB = 4.0  # subtracted from all scores via exp bias; cancels in softmax

F16 = mybir.dt.float16
F32 = mybir.dt.float32

# 19 key/value chunk offsets: 15 shifted (128c+64) + aligned 0,128,1792,1920
OFFS = [128 * c + 64 for c in range(15)] + [0, 128, 1792, 1920]


def _chunk_pair(i):
    if i == 0:
        return 15, 16
    if i == 15:
        return 17, 18
    return i - 1, i


def _build_pair_masks():
    # variant 0: (first, interior) — c4=0 pair 0
    # variant 1: (interior, interior)
    # variant 2: (interior, last)  — c4=3 pair 1
    m = _build_masks()  # [128, 3(first/int/last), 2(half), 128]
    mp = np.zeros((128, 3, 2, 2, 128), np.float16)
    mp[:, 0, 0] = m[:, 0]
    mp[:, 0, 1] = m[:, 1]
    mp[:, 1, 0] = m[:, 1]
    mp[:, 1, 1] = m[:, 1]
    mp[:, 2, 0] = m[:, 1]
    mp[:, 2, 1] = m[:, 2]
    return mp


def _build_masks():
    kp = np.arange(128)[:, None]
    p = np.arange(128)[None, :]
    masks = np.zeros((128, 3, 2, 128), np.float16)
    for v, shift in enumerate([0, 64, 128]):
        for half in (0, 1):
            w = 128 * half + kp
            valid = np.abs(p + shift - w) <= 64
            masks[:, v, half, :] = valid.astype(np.float16)
    return masks


def _build_program():
    nc = bacc.Bacc("TRN2", debug=False, num_devices=8)

    # xT packed host-side as [8 sb][128 kp][8 ko][256 si] (4KB lines)
    xT_d = nc.dram_tensor("xT", [8, 128, 8, 256], F16, kind="ExternalInput").ap()
    wqk_d = nc.dram_tensor("wqk", [128, 8, 512], F16, kind="ExternalInput").ap()
    wv_d = nc.dram_tensor("wv", [128, 8, 256], F16, kind="ExternalInput").ap()
    wout_d = nc.dram_tensor("wout", [128, 2, D], F16, kind="ExternalInput").ap()
    bqk_d = nc.dram_tensor("bqk", [128, 4], F32, kind="ExternalInput").ap()
    masks_d = nc.dram_tensor("masks", [128, 3, 2, 2, 128], F16,
                             kind="ExternalInput").ap()
    out_d = nc.dram_tensor("out", [S, D], F16, kind="ExternalOutput").ap()

    with tile.TileContext(nc) as tc:
        with (
            tc.tile_pool(name="const", bufs=1) as cpool,
            tc.tile_pool(name="expp", bufs=2) as epool,
            tc.tile_pool(name="bcsp", bufs=2) as bpool,
            tc.tile_pool(name="ysb", bufs=3) as ypool,
            tc.tile_pool(name="pshare", bufs=2, space="PSUM") as pshare,
            tc.tile_pool(name="pssc", bufs=1, space="PSUM") as pssc,
            tc.tile_pool(name="psav", bufs=2, space="PSUM") as psav,
            tc.tile_pool(name="psden", bufs=1, space="PSUM") as psden,
        ):
            # ---- persistent SBUF tensors ----
            # xT slice-major [sb, ko, si] so each 256-seq slice DMA lands
            # contiguously (4KB per-partition lines)
            xT_sb = cpool.tile([128, 8, 8, 256], F16, tag="xT")
            wqk_sb = cpool.tile([128, 8, 512], F16, tag="wqk")
            wv_sb = cpool.tile([128, 8, 256], F16, tag="wv")
            wout_sb = cpool.tile([128, 2, D], F16, tag="wout")
            bqk_sb = cpool.tile([128, 4], F32, tag="bqk")
            masks_sb = cpool.tile([128, 3, 2, 2, 128], F16, tag="masks")
            qk_sb = cpool.tile([128, 4, S], F16, tag="qk")
            v_sb = cpool.tile([128, 19, 4, 64], F16, tag="v")
            aoT_sb = cpool.tile([128, 2, S], F16, tag="aoT")
            ones_sb = cpool.tile([128, 128], F16, tag="ones")
            negc_sb = cpool.tile([128, 1], F32, tag="negc")

            # ---- input DMAs: single contiguous descriptors, priority order
            # (v-proj inputs first; <=8 in flight so order ~= arrival order)
            def dma_xt(sb):
                nc.sync.dma_start(out=xT_sb[:, sb], in_=xT_d[sb])

            nc.sync.dma_start(out=wv_sb[:], in_=wv_d)
            dma_xt(0)
            dma_xt(1)
            nc.sync.dma_start(out=bqk_sb[:], in_=bqk_d)
            nc.sync.dma_start(out=wqk_sb[:], in_=wqk_d)
            dma_xt(2)
            dma_xt(3)
            nc.sync.dma_start(out=masks_sb[:], in_=masks_d)
            for sb in range(4, 8):
                dma_xt(sb)
            nc.sync.dma_start(out=wout_sb[:], in_=wout_d)

            nc.vector.memset(ones_sb[:], 1.0)
            nc.vector.memset(negc_sb[:], -C_SUB)

            # ---- PE warmup: dummy matmuls on zeroed SBUF spanning the
            # input-DMA latency so HAM is at 8/8 when real work starts ----
            wsrc = cpool.tile([128, 512], F16, tag="wsrc")
            wdst = cpool.tile([128, 512], F16, tag="wdst")
            nc.vector.memset(wsrc[:], 0.0)
            wps = pshare.tile([128, 512], F32, tag="pshare")
            NWARM = 16
            for w in range(NWARM):
                nc.tensor.matmul(
                    out=wps[:],
                    lhsT=wsrc[:, 0:128],
                    rhs=wsrc[:],
                    start=(w == 0),
                    stop=(w == NWARM - 1),
                )
            nc.scalar.copy(out=wdst[:], in_=wps[:])

            # ---- emission helpers ----
            def emit_qk_m(ns, m):
                # one m-tile of q/k projection for seq chunk ns
                ps = pshare.tile([128, 512], F32, tag="pshare")
                for kt in range(8):
                    nc.tensor.matmul(
                        out=ps[:],
                        lhsT=wqk_sb[:, kt, m * 128:(m + 1) * 128],
                        rhs=xT_sb[:, 2 * ns:2 * ns + 2, kt, :],
                        start=(kt == 0),
                        stop=(kt == 7),
                    )
                nc.scalar.activation(
                    out=qk_sb[:, m, ns * 512:(ns + 1) * 512],
                    in_=ps[:],
                    func=mybir.ActivationFunctionType.Identity,
                    bias=bqk_sb[:, m:m + 1],
                )

            def emit_v_chunk(c):
                off = OFFS[c]
                sbi, o = divmod(off, 256)
                if o <= 128:
                    ps = pshare.tile([128, 512], F32, tag="pshare")
                    for kt in range(8):
                        nc.tensor.matmul(
                            out=ps[:, 0:256],
                            lhsT=xT_sb[:, sbi, kt, o:o + 128],
                            rhs=wv_sb[:, kt, :],
                            start=(kt == 0),
                            stop=(kt == 7),
                        )
                    nc.vector.tensor_copy(
                        out=v_sb[:, c, :, :],
                        in_=ps[:, 0:256].rearrange("p (h d) -> p h d", h=4),
                    )
                else:
                    # chunk straddles two slices: col-split into M=64 halves
                    # (PE col groups 0/1 vs 2/3 -> concurrent issue)
                    psa = pshare.tile([128, 512], F32, tag="pshare")
                    psb = pshare.tile([128, 512], F32, tag="pshare")
                    for kt in range(8):
                        nc.tensor.matmul(
                            out=psa[0:64, 0:256],
                            lhsT=xT_sb[:, sbi, kt, 192:256],
                            rhs=wv_sb[:, kt, :],
                            start=(kt == 0),
                            stop=(kt == 7),
                        )
                        nc.tensor.matmul(
                            out=psb[64:128, 0:256],
                            lhsT=xT_sb[:, sbi + 1, kt, 0:64],
                            rhs=wv_sb[:, kt, :],
                            start=(kt == 0),
                            stop=(kt == 7),
                        )
                    nc.vector.tensor_copy(
                        out=v_sb[0:64, c, :, :],
                        in_=psa[0:64, 0:256].rearrange("p (h d) -> p h d", h=4),
                    )
                    nc.vector.tensor_copy(
                        out=v_sb[64:128, c, :, :],
                        in_=psb[64:128, 0:256].rearrange("p (h d) -> p h d", h=4),
                    )

            def emit_scores_pair(c4, hp, pair, ex):
                # 8 score MMs for one query-tile pair, hh innermost so the
                # two heads (K=64 row groups 0/1 vs 2/3) issue back-to-back
                # and stream concurrently. One 2-bank sc tile per pair ->
                # a single merged exp op [128, 1024].
                if c4 == 0 and pair == 0:
                    pv = 0
                elif c4 == 3 and pair == 1:
                    pv = 2
                else:
                    pv = 1
                sc = pssc.tile([128, 2, 2, 2, 128], F32, tag="pssc")
                for iw in range(2):
                    i = c4 * 4 + pair * 2 + iw
                    cA, cB = _chunk_pair(i)
                    for half, cc in enumerate((cA, cB)):
                        off = OFFS[cc]
                        for hh in range(2):
                            po = hh * 64
                            nc.tensor.matmul(
                                out=sc[:, hh, iw, half, :],
                                lhsT=qk_sb[po:po + 64, 2 + hp,
                                           off:off + 128],
                                rhs=qk_sb[po:po + 64, hp,
                                          i * 128:(i + 1) * 128],
                                start=(iw == 0 and half == 0),
                                stop=(iw == 1 and half == 1),
                            )
                # exp(score - C) on ACT (one merged op per pair),
                # band-zeroing 0/1 mask on DVE (flat APs -> 2x fp16 mode)
                nc.scalar.activation(
                    out=ex[:, pair].rearrange("p a b c q -> p (a b c q)"),
                    in_=sc[:].rearrange("p a b c q -> p (a b c q)"),
                    func=mybir.ActivationFunctionType.Exp,
                    bias=negc_sb[:],
                )
                for hh in range(2):
                    nc.vector.tensor_mul(
                        out=ex[:, pair, hh].rearrange("p a b q -> p (a b q)"),
                        in0=ex[:, pair, hh].rearrange("p a b q -> p (a b q)"),
                        in1=masks_sb[:, pv].rearrange("p a b q -> p (a b q)"),
                    )

            def emit_av_den(c4, hp, ex, tailcb=None):
                # AV: two heads col-split (M=64 -> col groups 0/1 vs 2/3),
                # separate PSUM banks, hh innermost for concurrent issue.
                av0 = psav.tile([128, 4, 128], F32, tag="psav")
                av1 = psav.tile([128, 4, 128], F32, tag="psav")
                avs = {0: av0, 1: av1}
                for ii in range(4):
                    cA, cB = _chunk_pair(c4 * 4 + ii)
                    for half, cc in enumerate((cA, cB)):
                        for hh in range(2):
                            h = 2 * hp + hh
                            nc.tensor.matmul(
                                out=avs[hh][hh * 64:hh * 64 + 64, ii, :],
                                lhsT=v_sb[:, cc, h, :],
                                rhs=ex[:, ii // 2, hh, ii % 2, half, :],
                                start=(ii == 0 and half == 0),
                                stop=(ii == 3 and half == 1),
                            )
                # denominators: ones[128k,128m] @ ex -> den replicated on
                # all 128 partitions; one accumulation group per pair-bank.
                # Normalization runs per pair so tail consumers can start
                # after pair 0 (tailcb emits them between the pairs).
                den = psden.tile([128, 2, 2, 2, 128], F32, tag="psden")
                bcs = bpool.tile([128, 2, 2, 2, 128], F32, tag="bcs")
                for pair in range(2):
                    for half in range(2):
                        nc.tensor.matmul(
                            out=den[:, pair],
                            lhsT=ones_sb[:],
                            rhs=ex[:, pair, :, :, half, :],
                            start=(half == 0),
                            stop=(half == 1),
                        )
                if tailcb is None:
                    nc.vector.reciprocal_approx_fast(
                        out=bcs[:].rearrange("p a b c d -> p (a b c d)"),
                        in_=den[:].rearrange("p a b c d -> p (a b c d)"),
                    )
                    sl = slice(c4 * 512, (c4 + 1) * 512)
                    nc.vector.tensor_mul(
                        out=aoT_sb[0:64, hp, sl].rearrange(
                            "p (a b q) -> p a b q", a=2, b=2),
                        in0=av0[0:64, :, :].rearrange(
                            "p (a b) q -> p a b q", a=2),
                        in1=bcs[0:64, :, 0, :, :],
                    )
                    nc.vector.tensor_mul(
                        out=aoT_sb[64:128, hp, sl].rearrange(
                            "p (a b q) -> p a b q", a=2, b=2),
                        in0=av1[64:128, :, :].rearrange(
                            "p (a b) q -> p a b q", a=2),
                        in1=bcs[64:128, :, 1, :, :],
                    )
                else:
                    # pair-pipelined normalization so the tail outproj for
                    # pair 0's columns starts before pair 1 normalizes
                    for pair in range(2):
                        nc.vector.reciprocal_approx_fast(
                            out=bcs[:, pair].rearrange("p b c d -> p (b c d)"),
                            in_=den[:, pair].rearrange("p b c d -> p (b c d)"),
                        )
                        sl = slice(c4 * 512 + pair * 256,
                                   c4 * 512 + (pair + 1) * 256)
                        nc.vector.tensor_mul(
                            out=aoT_sb[0:64, hp, sl].rearrange(
                                "p (b q) -> p b q", b=2),
                            in0=av0[0:64, 2 * pair:2 * pair + 2, :],
                            in1=bcs[0:64, pair, 0, :, :],
                        )
                        nc.vector.tensor_mul(
                            out=aoT_sb[64:128, hp, sl].rearrange(
                                "p (b q) -> p b q", b=2),
                            in0=av1[64:128, 2 * pair:2 * pair + 2, :],
                            in1=bcs[64:128, pair, 1, :, :],
                        )
                        tailcb(pair)

            def _alt_psum():
                # extra tail rotation: pssc/psav banks are idle at the tail
                while True:
                    t_sc = pssc.tile([128, 2, 2, 2, 128], F32, tag="pssc")
                    yield t_sc.rearrange("p a b c q -> p (a b c q)")[:, 0:512]
                    t_av = psav.tile([128, 4, 128], F32, tag="psav")
                    yield t_av.rearrange("p a q -> p (a q)")

            def emit_outproj_st(st, alt=None):
                for nn in range(2):
                    if alt is not None and nn == 1:
                        ps = next(alt)
                    else:
                        ps = pshare.tile([128, 512], F32, tag="pshare")
                    for hp2 in range(2):
                        nc.tensor.matmul(
                            out=ps[:],
                            lhsT=aoT_sb[:, hp2, st * 128:(st + 1) * 128],
                            rhs=wout_sb[:, hp2, nn * 512:(nn + 1) * 512],
                            start=(hp2 == 0),
                            stop=(hp2 == 1),
                        )
                    ysb = ypool.tile([128, 512], F16, tag="ysb")
                    # evacuation split across ACT and DVE halves so the
                    # PSUM bank frees in ~0.4us instead of ~0.7us
                    nc.scalar.copy(out=ysb[:, 0:256], in_=ps[:, 0:256])
                    nc.vector.tensor_copy(out=ysb[:, 256:512],
                                          in_=ps[:, 256:512])
                    nc.sync.dma_start(
                        out=out_d[st * 128:(st + 1) * 128,
                                  nn * 512:(nn + 1) * 512],
                        in_=ysb[:],
                    )

            # ---- emission schedule ----
            # prologue: v chunks as the first xT slices land, then the qk
            # m-tiles block (0,0) needs; every block's scores inputs and AV
            # v-chunks are emitted as fillers of strictly earlier blocks.
            for c in (15, 16, 0, 1, 2):
                emit_v_chunk(c)
            for ns, m in ((0, 0), (0, 2), (1, 2)):
                emit_qk_m(ns, m)

            fillers = {
                (0, 0): (["q01", "v3"], ["q03", "q13", "v4"], []),
                (0, 1): (["q10"], ["q22"], ["v5", "v6", "q11"]),
                (1, 0): (["q23"], ["v7"], ["v8", "q20"]),
                (1, 1): (["q32"], ["v9"], ["v10", "q21"]),
                (2, 0): (["q33"], ["v11"], ["v12", "q30"]),
                (2, 1): (["s0", "s1"], ["v13"], ["v14", "v18", "q31"]),
                (3, 0): (["s2", "s3"], ["v17", "s4"], ["s5", "s6"]),
                (3, 1): (["s7", "s8"], ["s9", "s10", "s11"], []),
            }

            def emit_filler(f):
                kind, arg = f[0], f[1:]
                if kind == "v":
                    emit_v_chunk(int(arg))
                elif kind == "q":
                    emit_qk_m(int(arg[0]), int(arg[1]))
                else:
                    emit_outproj_st(int(arg))

            alt = _alt_psum()

            def tail_sts(pair):
                # outproj for the last block's freshly normalized columns
                for st in (12 + 2 * pair, 13 + 2 * pair):
                    emit_outproj_st(st, alt=alt)

            for c4 in range(4):
                for hp in range(2):
                    midA, midB, after = fillers[(c4, hp)]
                    ex = epool.tile([128, 2, 2, 2, 2, 128], F16, tag="exp")
                    emit_scores_pair(c4, hp, 0, ex)
                    for f in midA:
                        emit_filler(f)
                    emit_scores_pair(c4, hp, 1, ex)
                    for f in midB:
                        emit_filler(f)
                    emit_av_den(
                        c4, hp, ex,
                        tailcb=tail_sts if (c4, hp) == (3, 1) else None,
                    )
                    for f in after:
                        emit_filler(f)

    nc.compile()
    return nc


_NC = None


def _get_program():
    global _NC
    if _NC is None:
        _NC = _build_program()
    return _NC


def _make_in_maps(x, w_qkv, b_qkv, w_out):
    masks = _build_pair_masks()

    in_maps = []
    for c in range(8):
        b, hg = divmod(c, 4)
        cq = 256 * hg
        # q columns pre-scaled by SCALE (folded out of the activation)
        wqk = np.concatenate(
            [w_qkv[:, cq:cq + 256] * SCALE,
             w_qkv[:, 1024 + cq:1024 + cq + 256]],
            axis=1,
        ).astype(np.float16)          # [1024, 512]
        wqk_p = np.ascontiguousarray(
            wqk.reshape(8, 128, 512).transpose(1, 0, 2))
        wv = w_qkv[:, 2048 + cq:2048 + cq + 256].astype(np.float16)
        wv_p = np.ascontiguousarray(wv.reshape(8, 128, 256).transpose(1, 0, 2))
        wout_p = np.ascontiguousarray(
            w_out[cq:cq + 256, :].astype(np.float16)
            .reshape(2, 128, 1024).transpose(1, 0, 2))
        xb = x[b].astype(np.float16)  # [S, D]
        xT_p = np.ascontiguousarray(
            xb.reshape(8, 256, 8, 128).transpose(0, 3, 2, 1))
        bqk = np.empty((128, 4), np.float32)
        bqk[:, 0] = b_qkv[cq:cq + 128] * SCALE
        bqk[:, 1] = b_qkv[cq + 128:cq + 256] * SCALE
        bqk[:, 2] = b_qkv[1024 + cq:1024 + cq + 128]
        bqk[:, 3] = b_qkv[1024 + cq + 128:1024 + cq + 256]
        in_maps.append({
            "xT": xT_p,
            "wqk": wqk_p,
            "wv": wv_p,
            "wout": wout_p,
            "bqk": bqk,
            "masks": masks,
        })
    return in_maps


def kernel(x, w_qkv, b_qkv, w_out, b_out):
    x = np.asarray(x, np.float32)
    w_qkv = np.asarray(w_qkv, np.float32)
    b_qkv = np.asarray(b_qkv, np.float32)
    w_out = np.asarray(w_out, np.float32)
    b_out = np.asarray(b_out, np.float32)

    in_maps = _make_in_maps(x, w_qkv, b_qkv, w_out)
    nc = _get_program()
    res = run_bass_kernel_spmd(nc, in_maps, list(range(8)))

    b_v = b_qkv[2048:]
    bias_all = b_out + b_v @ w_out  # folds the (untracked) v-bias
    y = np.empty((B, S, D), np.float32)
    for b in range(B):
        acc = np.zeros((S, D), np.float32)
        for hg in range(4):
            acc += res.results[4 * b + hg]["out"].astype(np.float32)
        y[b] = acc + bias_all
    return y
